# revision 28
# baseline (speedup 1.0000x reference)
"""Trainium2 Bass kernel for nn_CaptionEmbedding (GRU caption embedding).

Strategy (see spec sharding_hint): pure data parallelism over the batch.
- Host sorts rows by cap_len descending, deals them round-robin to 8 cores
  (so per-core active-count profiles match within +-1), and bakes the
  per-timestep active column count A_t into the compiled program.
- Device layout is feature-on-partition / batch-on-free ("transposed"), so
  every matmul is stationary-weight x moving-batch with N = A_t columns.
- Everything runs in fp16 (weights, states, gates); PSUM accumulates fp32.
  Measured end-to-end error vs fp32 reference ~1.4e-3 relative.
- Outputs: att (alphas) and fc (pre-max outputs) stream out per step as
  packed fp16; host unpacks, masks padding, and takes the final max.
"""

import numpy as np

B, T, CD, HD, VD, QD = 2048, 20, 512, 1024, 2048, 1024
NCORES = 8
BC = B // NCORES  # 256 rows per core
P = 128
K1 = CD // P      # 4  k-tiles for 512-dim
K2 = HD // P      # 8  k-tiles for 1024-dim
KV = VD // P      # 16
KQ = QD // P      # 8
M1 = 3 * CD // P  # 12 M-tiles of GRU1 gates
M2 = 3 * HD // P  # 24 M-tiles of GRU2 gates
MF = HD // P      # 8  M-tiles of fc


def _build(A):
    """Build the Bass program for per-step active counts A (list of ints)."""
    import concourse.tile as tile
    from concourse import bacc, mybir

    f32, f16 = mybir.dt.float32, mybir.dt.float16
    AF = mybir.ActivationFunctionType

    Ts = len(A)
    Amax = max(A)
    xoff = np.concatenate([[0], np.cumsum([CD * a for a in A])])
    foff = np.concatenate([[0], np.cumsum([HD * a for a in A])])

    nc = bacc.Bacc("TRN2", target_bir_lowering=False, debug=False)

    d_vT = nc.dram_tensor("vT", [VD, BC], f16, kind="ExternalInput")
    d_qT = nc.dram_tensor("qT", [QD, BC], f16, kind="ExternalInput")
    d_wv = nc.dram_tensor("wvT", [VD, CD], f16, kind="ExternalInput")
    d_wq = nc.dram_tensor("wqT", [QD, CD], f16, kind="ExternalInput")
    d_wi1 = nc.dram_tensor("wi1T", [CD, 3 * CD], f16, kind="ExternalInput")
    d_wh1 = nc.dram_tensor("wh1T", [CD, 3 * CD], f16, kind="ExternalInput")
    d_wi2 = nc.dram_tensor("wi2T", [CD, 3 * HD], f16, kind="ExternalInput")
    d_wh2 = nc.dram_tensor("wh2T", [HD, 3 * HD], f16, kind="ExternalInput")
    d_wfc = nc.dram_tensor("wfcT", [HD, HD], f16, kind="ExternalInput")
    d_bias = nc.dram_tensor("biasall", [P, 48], f32, kind="ExternalInput")
    d_x = nc.dram_tensor("xpack", [int(xoff[-1])], f16, kind="ExternalInput")
    d_a = nc.dram_tensor("apack", [int(xoff[-1])], f16, kind="ExternalOutput")
    d_f = nc.dram_tensor("fpack", [int(foff[-1])], f16, kind="ExternalOutput")

    with tile.TileContext(nc) as tc:
        with tc.tile_pool(name="wpool", bufs=1) as wp, \
             tc.tile_pool(name="state", bufs=1) as sp, \
             tc.tile_pool(name="stream", bufs=2) as io, \
             tc.tile_pool(name="scratch", bufs=1) as sc, \
             tc.tile_pool(name="gps", bufs=7, space="PSUM") as gps, \
             tc.tile_pool(name="fps", bufs=1, space="PSUM") as fps:

            # ---- resident weights (order: pre-phase first; spread queues) ----
            _eng = [nc.gpsimd, nc.scalar, nc.sync]
            _ei = [0]

            def wload(name, dram, kt, cols):
                ts = []
                for k in range(kt):
                    t = wp.tile([P, cols], f16, name=f"{name}{k}")
                    _eng[_ei[0] % 3].dma_start(t[:], dram.ap()[k * P:(k + 1) * P, :])
                    _ei[0] += 1
                    ts.append(t)
                return ts

            ball = wp.tile([P, 48], f32, name="ball")
            nc.sync.dma_start(ball[:], d_bias.ap())
            b1rz, b1in, b1hn = ball[:, 0:8], ball[:, 8:12], ball[:, 12:16]
            b2rz, b2in, b2hn = ball[:, 16:32], ball[:, 32:40], ball[:, 40:48]

            xts = {}
            for tpre in range(min(2, Ts)):
                ap_ = A[tpre]
                xt_ = io.tile([P, K1 * Amax], f16, name=f"x{tpre}", tag="xt")
                nc.sync.dma_start(
                    xt_[:, :K1 * ap_].rearrange("p (k c) -> p k c", k=K1),
                    d_x.ap()[int(xoff[tpre]):int(xoff[tpre + 1])]
                        .rearrange("(k p c) -> p k c", k=K1, p=P, c=ap_))
                xts[tpre] = xt_

            wi1 = wload("wi1", d_wi1, K1, 3 * CD)
            wv = wload("wv", d_wv, KV, CD)
            wq = wload("wq", d_wq, KQ, CD)

            # ---- pre-phase: fvq = leaky(v@WvT) + leaky(q@WqT), transposed ----
            vsb = sp.tile([P, KV * BC], f16)
            nc.scalar.dma_start(
                vsb[:].rearrange("p (k a) -> p k a", k=KV, a=BC),
                d_vT.ap().rearrange("(k p) a -> p k a", k=KV, p=P))
            qsb = sp.tile([P, KQ * BC], f16)
            nc.gpsimd.dma_start(
                qsb[:].rearrange("p (k a) -> p k a", k=KQ, a=BC),
                d_qT.ap().rearrange("(k p) a -> p k a", k=KQ, p=P))
            wi2 = wload("wi2", d_wi2, K1, 3 * HD)
            wfc = wload("wfc", d_wfc, K2, HD)
            wh1 = wload("wh1", d_wh1, K1, 3 * CD)
            wh2 = wload("wh2", d_wh2, K2, 3 * HD)

            fvq = sp.tile([P, K1 * BC], f16)

            def compute_fvq():
                for m in range(K1):
                    ps = gps.tile([P, BC], f32, name=f"pv{m}", tag="g")
                    for k in range(KV):
                        nc.tensor.matmul(ps[:], wv[k][:, m * P:(m + 1) * P],
                                         vsb[:, k * BC:(k + 1) * BC],
                                         start=(k == 0), stop=(k == KV - 1))
                    nc.scalar.activation(fvq[:, m * BC:(m + 1) * BC], ps[:],
                                         AF.Lrelu, alpha=0.01)
                for m in range(K1):
                    ps = gps.tile([P, BC], f32, name=f"pq{m}", tag="g")
                    for k in range(KQ):
                        nc.tensor.matmul(ps[:], wq[k][:, m * P:(m + 1) * P],
                                         qsb[:, k * BC:(k + 1) * BC],
                                         start=(k == 0), stop=(k == KQ - 1))
                    fqm = sp.tile([P, BC], f16, name=f"fq{m}", tag="fqm")
                    nc.scalar.activation(fqm[:], ps[:], AF.Lrelu, alpha=0.01)
                    nc.vector.tensor_add(fvq[:, m * BC:(m + 1) * BC],
                                         fvq[:, m * BC:(m + 1) * BC], fqm[:])

            # ---- states ----
            h1 = sp.tile([P, K1 * BC], f16)
            nc.vector.memset(h1[:], 0.0)
            h2 = sp.tile([P, K2 * BC], f16)
            nc.vector.memset(h2[:], 0.0)

            def v3(ap2d, nk, a):  # [P, nk*BC] buffer -> [P, nk, a] active view
                return ap2d[:].rearrange("p (k c) -> p k c", k=nk)[:, :, :a]

            def p3(ap2d, nk, a):  # [P, nk*a] packed buffer -> [P, nk, a] view
                return ap2d[:, :nk * a].rearrange("p (k c) -> p k c", k=nk)

            # ---- time loop ----
            for t in range(Ts):
                a = A[t]
                if t in xts:
                    xt = xts.pop(t)
                else:
                    xt = io.tile([P, K1 * Amax], f16, name=f"x{t}", tag="xt")
                    nc.sync.dma_start(
                        p3(xt, K1, a),
                        d_x.ap()[int(xoff[t]):int(xoff[t + 1])]
                            .rearrange("(k p c) -> p k c", k=K1, p=P, c=a))

                # GRU1 r,z: psum = gi + gh, sigmoid with bias
                r1z1 = sc.tile([P, 8 * Amax], f16, name=f"rz1_{t}", tag="rz1")
                for m in range(8):
                    ps = gps.tile([P, Amax], f32, name=f"g1_{t}_{m}", tag="g")
                    for k in range(K1):
                        nc.tensor.matmul(ps[:, :a], wi1[k][:, m * P:(m + 1) * P],
                                         xt[:, k * a:(k + 1) * a],
                                         start=(k == 0),
                                         stop=(t == 0 and k == K1 - 1))
                    if t > 0:
                        for k in range(K1):
                            nc.tensor.matmul(ps[:, :a], wh1[k][:, m * P:(m + 1) * P],
                                             h1[:, k * BC:k * BC + a],
                                             start=False, stop=(k == K1 - 1))
                    nc.scalar.activation(r1z1[:, m * a:(m + 1) * a], ps[:, :a],
                                         AF.Sigmoid, bias=b1rz[:, m:m + 1])

                # GRU1 n: gin (bias b_ih1n), e = ghn + b_hh1n
                gin1 = sc.tile([P, 4 * Amax], f16, name=f"gin1_{t}", tag="gin1")
                e1 = sc.tile([P, 4 * Amax], f16, name=f"e1_{t}", tag="e1")
                for m in range(4):
                    ps = gps.tile([P, Amax], f32, name=f"n1i_{t}_{m}", tag="g")
                    for k in range(K1):
                        nc.tensor.matmul(ps[:, :a], wi1[k][:, (8 + m) * P:(9 + m) * P],
                                         xt[:, k * a:(k + 1) * a],
                                         start=(k == 0), stop=(k == K1 - 1))
                    nc.vector.tensor_scalar_add(gin1[:, m * a:(m + 1) * a],
                                                ps[:, :a], b1in[:, m:m + 1])
                    if t == 0:
                        nc.vector.tensor_scalar_add(e1[:, m * a:(m + 1) * a],
                                                    h1[:, :a], b1hn[:, m:m + 1])
                    else:
                        ps2 = gps.tile([P, Amax], f32, name=f"n1h_{t}_{m}", tag="g")
                        for k in range(K1):
                            nc.tensor.matmul(ps2[:, :a], wh1[k][:, (8 + m) * P:(9 + m) * P],
                                             h1[:, k * BC:k * BC + a],
                                             start=(k == 0), stop=(k == K1 - 1))
                        nc.vector.tensor_scalar_add(e1[:, m * a:(m + 1) * a], ps2[:, :a],
                                                    b1hn[:, m:m + 1])
                n1 = sc.tile([P, 4 * Amax], f16, name=f"n1_{t}", tag="n1")
                nc.vector.tensor_mul(n1[:, :4 * a], r1z1[:, :4 * a], e1[:, :4 * a])
                nc.vector.tensor_add(n1[:, :4 * a], n1[:, :4 * a], gin1[:, :4 * a])
                nc.scalar.activation(n1[:, :4 * a], n1[:, :4 * a], AF.Tanh)

                # h1 = n1 + z1*(h1 - n1)
                d1 = sc.tile([P, 4 * Amax], f16, name=f"d1_{t}", tag="gin1")
                nc.vector.tensor_sub(p3(d1, K1, a), v3(h1, K1, a), p3(n1, K1, a))
                nc.vector.tensor_mul(d1[:, :4 * a], r1z1[:, 4 * a:8 * a], d1[:, :4 * a])
                nc.vector.tensor_add(v3(h1, K1, a), p3(n1, K1, a), p3(d1, K1, a))

                # att = sigmoid(h1*fvq) * x
                if t == 0:
                    compute_fvq()
                att = io.tile([P, K1 * Amax], f16, name=f"att{t}", tag="att")
                nc.vector.tensor_mul(p3(att, K1, a), v3(h1, K1, a), v3(fvq, K1, a))
                nc.scalar.activation(att[:, :4 * a], att[:, :4 * a], AF.Sigmoid)
                nc.vector.tensor_mul(att[:, :4 * a], att[:, :4 * a], xt[:, :4 * a])
                nc.gpsimd.dma_start(
                    d_a.ap()[int(xoff[t]):int(xoff[t + 1])]
                        .rearrange("(k p c) -> p k c", k=K1, p=P, c=a),
                    p3(att, K1, a))

                # GRU2 r,z — gh2 first so these MMs are ready before att exists
                r2z2 = sc.tile([P, 16 * Amax], f16, name=f"rz2_{t}", tag="rz2")
                for m in range(16):
                    ps = gps.tile([P, Amax], f32, name=f"g2_{t}_{m}", tag="g")
                    if t > 0:
                        for k in range(K2):
                            nc.tensor.matmul(ps[:, :a], wh2[k][:, m * P:(m + 1) * P],
                                             h2[:, k * BC:k * BC + a],
                                             start=(k == 0), stop=False)
                    for k in range(K1):
                        nc.tensor.matmul(ps[:, :a], wi2[k][:, m * P:(m + 1) * P],
                                         att[:, k * a:(k + 1) * a],
                                         start=(t == 0 and k == 0),
                                         stop=(k == K1 - 1))
                    nc.scalar.activation(r2z2[:, m * a:(m + 1) * a], ps[:, :a],
                                         AF.Sigmoid, bias=b2rz[:, m:m + 1])

                # GRU2 n
                gin2 = sc.tile([P, 8 * Amax], f16, name=f"gin2_{t}", tag="gin2")
                e2 = sc.tile([P, 8 * Amax], f16, name=f"e2_{t}", tag="e2")
                for m in range(8):
                    ps = gps.tile([P, Amax], f32, name=f"n2i_{t}_{m}", tag="g")
                    for k in range(K1):
                        nc.tensor.matmul(ps[:, :a], wi2[k][:, (16 + m) * P:(17 + m) * P],
                                         att[:, k * a:(k + 1) * a],
                                         start=(k == 0), stop=(k == K1 - 1))
                    nc.vector.tensor_scalar_add(gin2[:, m * a:(m + 1) * a],
                                                ps[:, :a], b2in[:, m:m + 1])
                    if t == 0:
                        nc.vector.tensor_scalar_add(e2[:, m * a:(m + 1) * a],
                                                    h2[:, :a], b2hn[:, m:m + 1])
                    else:
                        ps2 = gps.tile([P, Amax], f32, name=f"n2h_{t}_{m}", tag="g")
                        for k in range(K2):
                            nc.tensor.matmul(ps2[:, :a], wh2[k][:, (16 + m) * P:(17 + m) * P],
                                             h2[:, k * BC:k * BC + a],
                                             start=(k == 0), stop=(k == K2 - 1))
                        nc.vector.tensor_scalar_add(e2[:, m * a:(m + 1) * a], ps2[:, :a],
                                                    b2hn[:, m:m + 1])
                n2 = sc.tile([P, 8 * Amax], f16, name=f"n2_{t}", tag="n2")
                nc.vector.tensor_mul(n2[:, :8 * a], r2z2[:, :8 * a], e2[:, :8 * a])
                nc.vector.tensor_add(n2[:, :8 * a], n2[:, :8 * a], gin2[:, :8 * a])
                nc.scalar.activation(n2[:, :8 * a], n2[:, :8 * a], AF.Tanh)

                d2 = sc.tile([P, 8 * Amax], f16, name=f"d2_{t}", tag="gin2")
                nc.vector.tensor_sub(p3(d2, K2, a), v3(h2, K2, a), p3(n2, K2, a))
                nc.vector.tensor_mul(d2[:, :8 * a], r2z2[:, 8 * a:16 * a], d2[:, :8 * a])
                nc.vector.tensor_add(v3(h2, K2, a), p3(n2, K2, a), p3(d2, K2, a))

                # fc = h2 @ WfcT  (leaky + mask + max done on host)
                fcb = io.tile([P, MF * Amax], f16, name=f"fc{t}", tag="fcb")
                for mp in range(MF // 2):
                    psf = fps.tile([P, 2 * Amax], f32, name=f"fps{t}_{mp}", tag="fc")
                    for half in range(2):
                        m = 2 * mp + half
                        for k in range(K2):
                            nc.tensor.matmul(psf[:, half * a:(half + 1) * a],
                                             wfc[k][:, m * P:(m + 1) * P],
                                             h2[:, k * BC:k * BC + a],
                                             start=(k == 0), stop=(k == K2 - 1))
                    nc.scalar.activation(fcb[:, 2 * mp * a:(2 * mp + 2) * a],
                                         psf[:, :2 * a], AF.Copy)
                nc.gpsimd.dma_start(
                    d_f.ap()[int(foff[t]):int(foff[t + 1])]
                        .rearrange("(k p c) -> p k c", k=MF, p=P, c=a),
                    p3(fcb, MF, a))

    nc.compile()
    return nc


def _schedule(L):
    """Sort rows by length desc, deal round-robin, derive per-step counts."""
    Teff = int(L.max()) if L.size else 0
    order = np.argsort(-L, kind="stable")
    rows = [order[c::NCORES] for c in range(NCORES)]
    G = np.array([(L > t).sum() for t in range(Teff)])
    A = [int(min(BC, -(-int(g) // NCORES) + (-(-int(g) // NCORES)) % 2))
         for g in G]
    cnt = np.array([[int((L[rows[c]] > t).sum()) for t in range(Teff)]
                    for c in range(NCORES)]) if Teff else np.zeros((NCORES, 0), int)
    return order, rows, A, cnt, Teff


def kernel(v, q, caption, cap_len, w_ih1, w_hh1, b_ih1, b_hh1,
           w_ih2, w_hh2, b_ih2, b_hh2, Wv, Wq, Wfc):
    from concourse.bass_utils import run_bass_kernel_spmd

    f16 = np.float16
    L = np.asarray(cap_len).astype(np.int64)
    out = np.zeros((B, HD), np.float32)
    alphas = np.zeros((B, T, CD), np.float32)

    order, rows, A, cnt, Teff = _schedule(L)
    if Teff <= 0:
        return out, alphas.astype(np.float32)

    nc = _build(A)

    # ---- pack inputs per core ----
    b1 = (np.asarray(b_ih1) + np.asarray(b_hh1)).astype(np.float32)
    b2 = (np.asarray(b_ih2) + np.asarray(b_hh2)).astype(np.float32)

    def pcol(vec, nm):  # [nm*128] -> [128, nm]
        return np.ascontiguousarray(vec.reshape(nm, P).T)

    shared = {
        "wvT": np.ascontiguousarray(np.asarray(Wv).T).astype(f16),
        "wqT": np.ascontiguousarray(np.asarray(Wq).T).astype(f16),
        "wi1T": np.ascontiguousarray(np.asarray(w_ih1).T).astype(f16),
        "wh1T": np.ascontiguousarray(np.asarray(w_hh1).T).astype(f16),
        "wi2T": np.ascontiguousarray(np.asarray(w_ih2).T).astype(f16),
        "wh2T": np.ascontiguousarray(np.asarray(w_hh2).T).astype(f16),
        "wfcT": np.ascontiguousarray(np.asarray(Wfc).T).astype(f16),
        "biasall": np.concatenate([
            pcol(b1[:2 * CD], 8),
            pcol(np.asarray(b_ih1)[2 * CD:].astype(np.float32), 4),
            pcol(np.asarray(b_hh1)[2 * CD:].astype(np.float32), 4),
            pcol(b2[:2 * HD], 16),
            pcol(np.asarray(b_ih2)[2 * HD:].astype(np.float32), 8),
            pcol(np.asarray(b_hh2)[2 * HD:].astype(np.float32), 8),
        ], axis=1),
    }

    vn, qn, capn = np.asarray(v), np.asarray(q), np.asarray(caption)
    in_maps = []
    for c in range(NCORES):
        r = rows[c]
        xp = np.concatenate([
            np.ascontiguousarray(capn[r[:A[t]], t, :].T).astype(f16).ravel()
            for t in range(Teff)])
        in_maps.append(dict(
            shared,
            vT=np.ascontiguousarray(vn[r].T).astype(f16),
            qT=np.ascontiguousarray(qn[r].T).astype(f16),
            xpack=xp,
        ))

    res = run_bass_kernel_spmd(nc, in_maps, core_ids=list(range(NCORES)))

    # ---- unpack ----
    xoff = np.concatenate([[0], np.cumsum([CD * a for a in A])]).astype(np.int64)
    foff = np.concatenate([[0], np.cumsum([HD * a for a in A])]).astype(np.int64)
    for c in range(NCORES):
        ap = res.results[c]["apack"]
        fp = res.results[c]["fpack"]
        r = rows[c]
        for t in range(Teff):
            a = A[t]
            n = int(cnt[c, t])
            if n == 0:
                continue
            blk = ap[xoff[t]:xoff[t + 1]].reshape(CD, a)[:, :n]
            alphas[r[:n], t, :] = blk.T.astype(np.float32)
            fblk = fp[foff[t]:foff[t + 1]].reshape(HD, a)[:, :n]
            rn = r[:n]
            out[rn] = np.maximum(out[rn], fblk.T.astype(np.float32))

    # zero out padded steps (also kills any polluted-column writes)
    mask = (np.arange(T)[None, :] < L[:, None])
    alphas *= mask[:, :, None]
    return out, alphas


# revision 29
# speedup vs baseline: 1.0206x; 1.0206x over previous
"""Trainium2 Bass kernel for nn_CaptionEmbedding (GRU caption embedding).

Strategy (see spec sharding_hint): pure data parallelism over the batch.
- Host sorts rows by cap_len descending, deals them round-robin to 8 cores
  (so per-core active-count profiles match within +-1), and bakes the
  per-timestep active column count A_t into the compiled program.
- Device layout is feature-on-partition / batch-on-free ("transposed"), so
  every matmul is stationary-weight x moving-batch with N = A_t columns.
- Everything runs in fp16 (weights, states, gates); PSUM accumulates fp32.
  Measured end-to-end error vs fp32 reference ~1.4e-3 relative.
- Outputs: att (alphas) and fc (pre-max outputs) stream out per step as
  packed fp16; host unpacks, masks padding, and takes the final max.
"""

import numpy as np

B, T, CD, HD, VD, QD = 2048, 20, 512, 1024, 2048, 1024
NCORES = 8
BC = B // NCORES  # 256 rows per core
P = 128
K1 = CD // P      # 4  k-tiles for 512-dim
K2 = HD // P      # 8  k-tiles for 1024-dim
KV = VD // P      # 16
KQ = QD // P      # 8
M1 = 3 * CD // P  # 12 M-tiles of GRU1 gates
M2 = 3 * HD // P  # 24 M-tiles of GRU2 gates
MF = HD // P      # 8  M-tiles of fc


def _build(A):
    """Build the Bass program for per-step active counts A (list of ints)."""
    import concourse.tile as tile
    from concourse import bacc, mybir

    f32, f16 = mybir.dt.float32, mybir.dt.float16
    AF = mybir.ActivationFunctionType

    Ts = len(A)
    Amax = max(A)
    xoff = np.concatenate([[0], np.cumsum([CD * a for a in A])])
    foff = np.concatenate([[0], np.cumsum([HD * a for a in A])])

    nc = bacc.Bacc("TRN2", target_bir_lowering=False, debug=False)

    d_vT = nc.dram_tensor("vT", [VD, BC], f16, kind="ExternalInput")
    d_qT = nc.dram_tensor("qT", [QD, BC], f16, kind="ExternalInput")
    d_wv = nc.dram_tensor("wvT", [VD, CD], f16, kind="ExternalInput")
    d_wq = nc.dram_tensor("wqT", [QD, CD], f16, kind="ExternalInput")
    d_wi1 = nc.dram_tensor("wi1T", [CD, 3 * CD], f16, kind="ExternalInput")
    d_wh1 = nc.dram_tensor("wh1T", [CD, 3 * CD], f16, kind="ExternalInput")
    d_wi2 = nc.dram_tensor("wi2T", [CD, 3 * HD], f16, kind="ExternalInput")
    d_wh2 = nc.dram_tensor("wh2T", [HD, 3 * HD], f16, kind="ExternalInput")
    d_wfc = nc.dram_tensor("wfcT", [HD, HD], f16, kind="ExternalInput")
    d_bias = nc.dram_tensor("biasall", [P, 48], f32, kind="ExternalInput")
    d_x = nc.dram_tensor("xpack", [int(xoff[-1])], f16, kind="ExternalInput")
    d_a = nc.dram_tensor("apack", [int(xoff[-1])], f16, kind="ExternalOutput")
    d_f = nc.dram_tensor("fpack", [int(foff[-1])], f16, kind="ExternalOutput")

    with tile.TileContext(nc) as tc:
        with tc.tile_pool(name="wpool", bufs=1) as wp, \
             tc.tile_pool(name="state", bufs=1) as sp, \
             tc.tile_pool(name="stream", bufs=2) as io, \
             tc.tile_pool(name="scratch", bufs=1) as sc, \
             tc.tile_pool(name="gps", bufs=7, space="PSUM") as gps, \
             tc.tile_pool(name="fps", bufs=1, space="PSUM") as fps:

            # ---- resident weights (order: pre-phase first; spread queues) ----
            _eng = [nc.gpsimd, nc.sync]
            _ei = [0]

            def wload(name, dram, kt, cols):
                ts = []
                for k in range(kt):
                    t = wp.tile([P, cols], f16, name=f"{name}{k}")
                    _eng[_ei[0] % 2].dma_start(t[:], dram.ap()[k * P:(k + 1) * P, :])
                    _ei[0] += 1
                    ts.append(t)
                return ts

            ball = wp.tile([P, 48], f32, name="ball")
            nc.sync.dma_start(ball[:], d_bias.ap())
            b1rz, b1in, b1hn = ball[:, 0:8], ball[:, 8:12], ball[:, 12:16]
            b2rz, b2in, b2hn = ball[:, 16:32], ball[:, 32:40], ball[:, 40:48]

            xts = {}
            for tpre in range(min(2, Ts)):
                ap_ = A[tpre]
                xt_ = io.tile([P, K1 * Amax], f16, name=f"x{tpre}", tag="xt")
                nc.sync.dma_start(
                    xt_[:, :K1 * ap_].rearrange("p (k c) -> p k c", k=K1),
                    d_x.ap()[int(xoff[tpre]):int(xoff[tpre + 1])]
                        .rearrange("(k p c) -> p k c", k=K1, p=P, c=ap_))
                xts[tpre] = xt_

            wi1 = wload("wi1", d_wi1, K1, 3 * CD)
            wv = wload("wv", d_wv, KV, CD)
            wq = wload("wq", d_wq, KQ, CD)

            # ---- pre-phase: fvq = leaky(v@WvT) + leaky(q@WqT), transposed ----
            vsb = sp.tile([P, KV * BC], f16)
            nc.sync.dma_start(
                vsb[:].rearrange("p (k a) -> p k a", k=KV, a=BC),
                d_vT.ap().rearrange("(k p) a -> p k a", k=KV, p=P))
            qsb = sp.tile([P, KQ * BC], f16)
            nc.gpsimd.dma_start(
                qsb[:].rearrange("p (k a) -> p k a", k=KQ, a=BC),
                d_qT.ap().rearrange("(k p) a -> p k a", k=KQ, p=P))
            wi2 = wload("wi2", d_wi2, K1, 3 * HD)
            wfc = wload("wfc", d_wfc, K2, HD)
            wh1 = wload("wh1", d_wh1, K1, 3 * CD)
            wh2 = wload("wh2", d_wh2, K2, 3 * HD)

            fvq = sp.tile([P, K1 * BC], f16)

            def compute_fvq():
                for m in range(K1):
                    ps = gps.tile([P, BC], f32, name=f"pv{m}", tag="g")
                    for k in range(KV):
                        nc.tensor.matmul(ps[:], wv[k][:, m * P:(m + 1) * P],
                                         vsb[:, k * BC:(k + 1) * BC],
                                         start=(k == 0), stop=(k == KV - 1))
                    nc.scalar.activation(fvq[:, m * BC:(m + 1) * BC], ps[:],
                                         AF.Lrelu, alpha=0.01)
                for m in range(K1):
                    ps = gps.tile([P, BC], f32, name=f"pq{m}", tag="g")
                    for k in range(KQ):
                        nc.tensor.matmul(ps[:], wq[k][:, m * P:(m + 1) * P],
                                         qsb[:, k * BC:(k + 1) * BC],
                                         start=(k == 0), stop=(k == KQ - 1))
                    fqm = sp.tile([P, BC], f16, name=f"fq{m}", tag="fqm")
                    nc.scalar.activation(fqm[:], ps[:], AF.Lrelu, alpha=0.01)
                    nc.vector.tensor_add(fvq[:, m * BC:(m + 1) * BC],
                                         fvq[:, m * BC:(m + 1) * BC], fqm[:])

            # ---- states ----
            h1 = sp.tile([P, K1 * BC], f16)
            nc.vector.memset(h1[:], 0.0)
            h2 = sp.tile([P, K2 * BC], f16)
            nc.vector.memset(h2[:], 0.0)

            def v3(ap2d, nk, a):  # [P, nk*BC] buffer -> [P, nk, a] active view
                return ap2d[:].rearrange("p (k c) -> p k c", k=nk)[:, :, :a]

            def p3(ap2d, nk, a):  # [P, nk*a] packed buffer -> [P, nk, a] view
                return ap2d[:, :nk * a].rearrange("p (k c) -> p k c", k=nk)

            # ---- time loop ----
            for t in range(Ts):
                a = A[t]
                if t in xts:
                    xt = xts.pop(t)
                else:
                    xt = io.tile([P, K1 * Amax], f16, name=f"x{t}", tag="xt")
                    nc.sync.dma_start(
                        p3(xt, K1, a),
                        d_x.ap()[int(xoff[t]):int(xoff[t + 1])]
                            .rearrange("(k p c) -> p k c", k=K1, p=P, c=a))

                # GRU1 r,z: psum = gi + gh, sigmoid with bias
                r1z1 = sc.tile([P, 8 * Amax], f16, name=f"rz1_{t}", tag="rz1")
                for m in range(8):
                    ps = gps.tile([P, Amax], f32, name=f"g1_{t}_{m}", tag="g")
                    for k in range(K1):
                        nc.tensor.matmul(ps[:, :a], wi1[k][:, m * P:(m + 1) * P],
                                         xt[:, k * a:(k + 1) * a],
                                         start=(k == 0),
                                         stop=(t == 0 and k == K1 - 1))
                    if t > 0:
                        for k in range(K1):
                            nc.tensor.matmul(ps[:, :a], wh1[k][:, m * P:(m + 1) * P],
                                             h1[:, k * BC:k * BC + a],
                                             start=False, stop=(k == K1 - 1))
                    nc.scalar.activation(r1z1[:, m * a:(m + 1) * a], ps[:, :a],
                                         AF.Sigmoid, bias=b1rz[:, m:m + 1])

                # GRU1 n: gin (bias b_ih1n), e = ghn + b_hh1n
                gin1 = sc.tile([P, 4 * Amax], f16, name=f"gin1_{t}", tag="gin1")
                e1 = sc.tile([P, 4 * Amax], f16, name=f"e1_{t}", tag="e1")
                for m in range(4):
                    ps = gps.tile([P, Amax], f32, name=f"n1i_{t}_{m}", tag="g")
                    for k in range(K1):
                        nc.tensor.matmul(ps[:, :a], wi1[k][:, (8 + m) * P:(9 + m) * P],
                                         xt[:, k * a:(k + 1) * a],
                                         start=(k == 0), stop=(k == K1 - 1))
                    nc.vector.tensor_scalar_add(gin1[:, m * a:(m + 1) * a],
                                                ps[:, :a], b1in[:, m:m + 1])
                    if t == 0:
                        nc.vector.tensor_scalar_add(e1[:, m * a:(m + 1) * a],
                                                    h1[:, :a], b1hn[:, m:m + 1])
                    else:
                        ps2 = gps.tile([P, Amax], f32, name=f"n1h_{t}_{m}", tag="g")
                        for k in range(K1):
                            nc.tensor.matmul(ps2[:, :a], wh1[k][:, (8 + m) * P:(9 + m) * P],
                                             h1[:, k * BC:k * BC + a],
                                             start=(k == 0), stop=(k == K1 - 1))
                        nc.vector.tensor_scalar_add(e1[:, m * a:(m + 1) * a], ps2[:, :a],
                                                    b1hn[:, m:m + 1])
                n1 = sc.tile([P, 4 * Amax], f16, name=f"n1_{t}", tag="n1")
                nc.vector.tensor_mul(n1[:, :4 * a], r1z1[:, :4 * a], e1[:, :4 * a])
                nc.vector.tensor_add(n1[:, :4 * a], n1[:, :4 * a], gin1[:, :4 * a])
                nc.scalar.activation(n1[:, :4 * a], n1[:, :4 * a], AF.Tanh)

                # h1 = n1 + z1*(h1 - n1)
                d1 = sc.tile([P, 4 * Amax], f16, name=f"d1_{t}", tag="gin1")
                nc.vector.tensor_sub(p3(d1, K1, a), v3(h1, K1, a), p3(n1, K1, a))
                nc.vector.tensor_mul(d1[:, :4 * a], r1z1[:, 4 * a:8 * a], d1[:, :4 * a])
                nc.vector.tensor_add(v3(h1, K1, a), p3(n1, K1, a), p3(d1, K1, a))

                # att = sigmoid(h1*fvq) * x
                if t == 0:
                    compute_fvq()
                att = io.tile([P, K1 * Amax], f16, name=f"att{t}", tag="att")
                nc.vector.tensor_mul(p3(att, K1, a), v3(h1, K1, a), v3(fvq, K1, a))
                nc.scalar.activation(att[:, :4 * a], att[:, :4 * a], AF.Sigmoid)
                nc.vector.tensor_mul(att[:, :4 * a], att[:, :4 * a], xt[:, :4 * a])
                nc.gpsimd.dma_start(
                    d_a.ap()[int(xoff[t]):int(xoff[t + 1])]
                        .rearrange("(k p c) -> p k c", k=K1, p=P, c=a),
                    p3(att, K1, a))

                # GRU2 r,z — gh2 first so these MMs are ready before att exists
                r2z2 = sc.tile([P, 16 * Amax], f16, name=f"rz2_{t}", tag="rz2")
                for m in range(16):
                    ps = gps.tile([P, Amax], f32, name=f"g2_{t}_{m}", tag="g")
                    if t > 0:
                        for k in range(K2):
                            nc.tensor.matmul(ps[:, :a], wh2[k][:, m * P:(m + 1) * P],
                                             h2[:, k * BC:k * BC + a],
                                             start=(k == 0), stop=False)
                    for k in range(K1):
                        nc.tensor.matmul(ps[:, :a], wi2[k][:, m * P:(m + 1) * P],
                                         att[:, k * a:(k + 1) * a],
                                         start=(t == 0 and k == 0),
                                         stop=(k == K1 - 1))
                    nc.scalar.activation(r2z2[:, m * a:(m + 1) * a], ps[:, :a],
                                         AF.Sigmoid, bias=b2rz[:, m:m + 1])

                # GRU2 n
                gin2 = sc.tile([P, 8 * Amax], f16, name=f"gin2_{t}", tag="gin2")
                e2 = sc.tile([P, 8 * Amax], f16, name=f"e2_{t}", tag="e2")
                for m in range(8):
                    ps = gps.tile([P, Amax], f32, name=f"n2i_{t}_{m}", tag="g")
                    for k in range(K1):
                        nc.tensor.matmul(ps[:, :a], wi2[k][:, (16 + m) * P:(17 + m) * P],
                                         att[:, k * a:(k + 1) * a],
                                         start=(k == 0), stop=(k == K1 - 1))
                    nc.vector.tensor_scalar_add(gin2[:, m * a:(m + 1) * a],
                                                ps[:, :a], b2in[:, m:m + 1])
                    if t == 0:
                        nc.vector.tensor_scalar_add(e2[:, m * a:(m + 1) * a],
                                                    h2[:, :a], b2hn[:, m:m + 1])
                    else:
                        ps2 = gps.tile([P, Amax], f32, name=f"n2h_{t}_{m}", tag="g")
                        for k in range(K2):
                            nc.tensor.matmul(ps2[:, :a], wh2[k][:, (16 + m) * P:(17 + m) * P],
                                             h2[:, k * BC:k * BC + a],
                                             start=(k == 0), stop=(k == K2 - 1))
                        nc.vector.tensor_scalar_add(e2[:, m * a:(m + 1) * a], ps2[:, :a],
                                                    b2hn[:, m:m + 1])
                n2 = sc.tile([P, 8 * Amax], f16, name=f"n2_{t}", tag="n2")
                nc.vector.tensor_mul(n2[:, :8 * a], r2z2[:, :8 * a], e2[:, :8 * a])
                nc.vector.tensor_add(n2[:, :8 * a], n2[:, :8 * a], gin2[:, :8 * a])
                nc.scalar.activation(n2[:, :8 * a], n2[:, :8 * a], AF.Tanh)

                d2 = sc.tile([P, 8 * Amax], f16, name=f"d2_{t}", tag="gin2")
                nc.vector.tensor_sub(p3(d2, K2, a), v3(h2, K2, a), p3(n2, K2, a))
                nc.vector.tensor_mul(d2[:, :8 * a], r2z2[:, 8 * a:16 * a], d2[:, :8 * a])
                nc.vector.tensor_add(v3(h2, K2, a), p3(n2, K2, a), p3(d2, K2, a))

                # fc = h2 @ WfcT  (leaky + mask + max done on host)
                fcb = io.tile([P, MF * Amax], f16, name=f"fc{t}", tag="fcb")
                for mp in range(MF // 2):
                    psf = fps.tile([P, 2 * Amax], f32, name=f"fps{t}_{mp}", tag="fc")
                    for half in range(2):
                        m = 2 * mp + half
                        for k in range(K2):
                            nc.tensor.matmul(psf[:, half * a:(half + 1) * a],
                                             wfc[k][:, m * P:(m + 1) * P],
                                             h2[:, k * BC:k * BC + a],
                                             start=(k == 0), stop=(k == K2 - 1))
                    nc.scalar.activation(fcb[:, 2 * mp * a:(2 * mp + 2) * a],
                                         psf[:, :2 * a], AF.Copy)
                nc.gpsimd.dma_start(
                    d_f.ap()[int(foff[t]):int(foff[t + 1])]
                        .rearrange("(k p c) -> p k c", k=MF, p=P, c=a),
                    p3(fcb, MF, a))

    nc.compile()
    return nc


def _schedule(L):
    """Sort rows by length desc, deal round-robin, derive per-step counts."""
    Teff = int(L.max()) if L.size else 0
    order = np.argsort(-L, kind="stable")
    rows = [order[c::NCORES] for c in range(NCORES)]
    G = np.array([(L > t).sum() for t in range(Teff)])
    A = [int(min(BC, -(-int(g) // NCORES) + (-(-int(g) // NCORES)) % 2))
         for g in G]
    cnt = np.array([[int((L[rows[c]] > t).sum()) for t in range(Teff)]
                    for c in range(NCORES)]) if Teff else np.zeros((NCORES, 0), int)
    return order, rows, A, cnt, Teff


def kernel(v, q, caption, cap_len, w_ih1, w_hh1, b_ih1, b_hh1,
           w_ih2, w_hh2, b_ih2, b_hh2, Wv, Wq, Wfc):
    from concourse.bass_utils import run_bass_kernel_spmd

    f16 = np.float16
    L = np.asarray(cap_len).astype(np.int64)
    out = np.zeros((B, HD), np.float32)
    alphas = np.zeros((B, T, CD), np.float32)

    order, rows, A, cnt, Teff = _schedule(L)
    if Teff <= 0:
        return out, alphas.astype(np.float32)

    nc = _build(A)

    # ---- pack inputs per core ----
    b1 = (np.asarray(b_ih1) + np.asarray(b_hh1)).astype(np.float32)
    b2 = (np.asarray(b_ih2) + np.asarray(b_hh2)).astype(np.float32)

    def pcol(vec, nm):  # [nm*128] -> [128, nm]
        return np.ascontiguousarray(vec.reshape(nm, P).T)

    shared = {
        "wvT": np.ascontiguousarray(np.asarray(Wv).T).astype(f16),
        "wqT": np.ascontiguousarray(np.asarray(Wq).T).astype(f16),
        "wi1T": np.ascontiguousarray(np.asarray(w_ih1).T).astype(f16),
        "wh1T": np.ascontiguousarray(np.asarray(w_hh1).T).astype(f16),
        "wi2T": np.ascontiguousarray(np.asarray(w_ih2).T).astype(f16),
        "wh2T": np.ascontiguousarray(np.asarray(w_hh2).T).astype(f16),
        "wfcT": np.ascontiguousarray(np.asarray(Wfc).T).astype(f16),
        "biasall": np.concatenate([
            pcol(b1[:2 * CD], 8),
            pcol(np.asarray(b_ih1)[2 * CD:].astype(np.float32), 4),
            pcol(np.asarray(b_hh1)[2 * CD:].astype(np.float32), 4),
            pcol(b2[:2 * HD], 16),
            pcol(np.asarray(b_ih2)[2 * HD:].astype(np.float32), 8),
            pcol(np.asarray(b_hh2)[2 * HD:].astype(np.float32), 8),
        ], axis=1),
    }

    vn, qn, capn = np.asarray(v), np.asarray(q), np.asarray(caption)
    in_maps = []
    for c in range(NCORES):
        r = rows[c]
        xp = np.concatenate([
            np.ascontiguousarray(capn[r[:A[t]], t, :].T).astype(f16).ravel()
            for t in range(Teff)])
        in_maps.append(dict(
            shared,
            vT=np.ascontiguousarray(vn[r].T).astype(f16),
            qT=np.ascontiguousarray(qn[r].T).astype(f16),
            xpack=xp,
        ))

    res = run_bass_kernel_spmd(nc, in_maps, core_ids=list(range(NCORES)))

    # ---- unpack ----
    xoff = np.concatenate([[0], np.cumsum([CD * a for a in A])]).astype(np.int64)
    foff = np.concatenate([[0], np.cumsum([HD * a for a in A])]).astype(np.int64)
    for c in range(NCORES):
        ap = res.results[c]["apack"]
        fp = res.results[c]["fpack"]
        r = rows[c]
        for t in range(Teff):
            a = A[t]
            n = int(cnt[c, t])
            if n == 0:
                continue
            blk = ap[xoff[t]:xoff[t + 1]].reshape(CD, a)[:, :n]
            alphas[r[:n], t, :] = blk.T.astype(np.float32)
            fblk = fp[foff[t]:foff[t + 1]].reshape(HD, a)[:, :n]
            rn = r[:n]
            out[rn] = np.maximum(out[rn], fblk.T.astype(np.float32))

    # zero out padded steps (also kills any polluted-column writes)
    mask = (np.arange(T)[None, :] < L[:, None])
    alphas *= mask[:, :, None]
    return out, alphas
